# revision 34
# baseline (speedup 1.0000x reference)
"""Trainium2 Bass kernel for nn_BertCounterFactTransformer.

Contract: kernel(**inputs) takes FULL unsharded numpy inputs (as produced by
reference.setup_inputs()) and returns the FULL [32, 1024] float32 output.

Strategy (data-parallel over batch, 8 cores x 4 samples):
  - Host: compute sep positions from x_ids, SORT samples by sep position and
    assign sorted rank r -> core (r % 8), slot (r // 8) so the per-slot-pair
    tile bounds are tight and uniform across cores. Precompute
    M_p = W_pq @ W_pk^T, pre-scaled x64 into fp8 e4m3 normal range, so
    scores are x M x^T (only the smaller side is ever projected). x is
    packed per-core into fp8 xq (false-row cols) / xo (option cols) plus a
    bf16 xqb copy for the precision-critical gate.
  - Device, per pair of slots (F tiles of false rows, option cols from OJ):
      gate     all-4-sample anomaly logits in ONE [4, wg] PSUM (bf16) via
               block-diagonal embedded w_anom; false-mask folded as bias
               rows; exp+normalize row-wise; PE-transpose to columns
      proj     qT = (x M_p)^T (or M_p x_opt^T if the option side is
               smaller) as fp8 DoubleRow chains, 2 k-tiles per matmul and
               2 samples per rhs (width<=512)
      scores   S = q x_opt^T as fp8 DoubleRow chains; option mask via
               bias rows into PSUM; exp/tanh scales fold the 1/64
      E_sup = exp(S_sup/32 + ob), E_rep = exp(S_rep/32 + tanh(S_con/32) + ob)
      coeff_t = gate / rowsum(E_t);  r_t = E_t^T @ coeff_t  (width-1 chains)
  - Pool: ONE [12, 512] PSUM accumulates x_s^T @ [gate|r_rep|r_sup] for all
    4 samples via 12-col G with per-sample zero blocks; PE-transpose the
    [12, 1024] result into fused^T columns.
  - Tail in row form: h = relu(fused @ W1 + b1), y = h @ W2 + b2, LayerNorm
    along the free dim (skipping the affine when ln_g==1, ln_b==0), split
    [4, 512] output DMAs. W1's first half is prefetched early when SBUF
    allows so the tail never waits on HBM.

Key identity: gate @ (attn @ x) == (gate @ attn) @ x, so [L,D] attention
outputs are never materialized. Nonzero projection biases fall back to an
exact numpy path (not expected in practice).
"""

import sys

if "/opt/trn_rl_repo" not in sys.path:
    sys.path.insert(0, "/opt/trn_rl_repo")

import numpy as np
import ml_dtypes
from contextlib import ExitStack

np_bf16 = ml_dtypes.bfloat16
np_fp8 = ml_dtypes.float8_e4m3

import concourse.bacc as bacc
import concourse.bass as bass
import concourse.mybir as mybir
import concourse.tile as tile
from concourse import bass_utils

f32 = mybir.dt.float32
bf16 = mybir.dt.bfloat16
fp8 = mybir.dt.float8e4
AF = mybir.ActivationFunctionType
ALU = mybir.AluOpType

B, L, D = 32, 512, 1024
NCORES = 8
BC = B // NCORES          # samples per core
NL = L // 128             # 4 L-tiles
ND = D // 128             # 8 D-tiles
NC3 = 3 * D // 128        # 24 tiles of the 3D fused dim
SCALE = 1.0 / 32.0        # 1/sqrt(D)
OBIAS_RAW = -960.0        # -30 after * SCALE
FBIAS = -30.0
LN_EPS = 1e-5
USE_FP8 = True            # fp8 e4m3 + DoubleRow for the projection GEMMs
FP8_SCORES = True         # score operands (projT, xo) also fp8 + DoubleRow
PRE = 64.0 if USE_FP8 else 1.0   # pre-scale on M/w_anom (fp8 normal range)

_PROGRAM_CACHE = {}
_M_CACHE = {}


def _m_matrix(wq, wk, transposed=False):
    import hashlib
    wq = np.asarray(wq, dtype=np.float32)
    wk = np.asarray(wk, dtype=np.float32)
    key = (hashlib.blake2b(wq.tobytes() + wk.tobytes(), digest_size=16).digest(),
           transposed, USE_FP8)
    if key not in _M_CACHE:
        m = wq @ wk.T
        if transposed:
            m = m.T
        m = np.ascontiguousarray(m)
        if USE_FP8:
            _M_CACHE[key] = np.clip(m * PRE, -240, 240).astype(np_fp8)
        else:
            _M_CACHE[key] = m.astype(np_bf16)
    return _M_CACHE[key]


def _geo(F, J0):
    OJ = J0 * 128
    NO = L - OJ
    CQ = F * 128
    have = NO > 0
    side_q = (CQ <= NO) if have else True
    w = (CQ if side_q else NO) if have else 0
    lo = 0 if side_q else OJ
    return dict(F=F, J0=J0, OJ=OJ, NO=NO, CQ=CQ, have=have,
                side_q=side_q, w=w, lo=lo)


def build_program_fast(pair_geo, ln_trivial=False):
    """pair_geo = ((F0, J0_0), (F1, J0_1)); pair p covers slots {2p, 2p+1}.
    Computing a superset is always correct (bias masks zero it)."""
    nc = bacc.Bacc(
        "TRN2",
        target_bir_lowering=False,
        debug=False,
        enable_asserts=False,
        num_devices=NCORES,
    )

    geos = [_geo(F, J0) for (F, J0) in pair_geo]
    need_mt = any(g["have"] and not g["side_q"] for g in geos)
    wg = max(max(g["CQ"] for g in geos), 128)       # gate width (cols 0..wg)
    maxF = max(g["F"] for g in geos)
    DT_X = fp8 if USE_FP8 else bf16
    DT_SC = fp8 if (USE_FP8 and FP8_SCORES) else bf16
    need_xqb = USE_FP8             # gate chains always run bf16
    CQs = [geos[s // 2]["CQ"] for s in range(BC)]
    NOs = [geos[s // 2]["NO"] for s in range(BC)]
    offq = [sum(CQs[:s]) for s in range(BC)]
    offo = [sum(NOs[:s]) for s in range(BC)]
    SQ, SO = sum(CQs), sum(NOs)

    # per-partition SBUF estimate (bytes) with w1-half0 resident early
    _xb = 1 if USE_FP8 else 2
    _scb = 1 if (USE_FP8 and FP8_SCORES) else 2
    _est = (ND * SQ * _xb + ND * SO * _scb + BC * NL * D * 2     # xq, xo, x
            + (ND * SQ * 2 if USE_FP8 else 0)                    # xqb
            + 3 * ND * D * _xb * (2 if need_mt else 1)           # M (+MT)
            + max(ND * 2 * g["w"] * _scb for g in geos) * 3      # projT
            + max(4 * g["F"] * g["NO"] * 2 for g in geos)        # E (bf16)
            + NC3 * 512 * 2                                      # w1 half 0
            + 60 * 1024)                                         # misc + slack
    early_tail = _est <= 200 * 1024
    _projT_extra = (sum(ND * 2 * g["w"] * _scb for g in geos)
                    - max(ND * 2 * g["w"] * _scb for g in geos)) * 3
    interleave_proj = (_est + _projT_extra) <= 200 * 1024

    xq_d = nc.dram_tensor("xq", [ND, 128, SQ], DT_X, kind="ExternalInput").ap()
    xqb_d = (nc.dram_tensor("xqb", [ND, 128, SQ], bf16,
                            kind="ExternalInput").ap() if need_xqb else None)
    need_xo8 = (DT_SC != DT_X) and any(
        g["have"] and not g["side_q"] for g in geos)
    xo_d = (nc.dram_tensor("xo", [ND, 128, SO], DT_SC, kind="ExternalInput").ap()
            if SO else None)
    xo8_d = (nc.dram_tensor("xo8", [ND, 128, SO], DT_X,
                            kind="ExternalInput").ap()
             if (SO and need_xo8) else None)
    x_d = nc.dram_tensor("x", [BC, L, D], bf16, kind="ExternalInput").ap()
    fb_d = nc.dram_tensor("fbias", [BC, L], bf16, kind="ExternalInput").ap()
    ob_d = nc.dram_tensor("obias", [BC, L], bf16, kind="ExternalInput").ap()
    m_d = [nc.dram_tensor(f"m{p}", [D, D], DT_X, kind="ExternalInput").ap()
           for p in range(3)]
    mt_d = ([nc.dram_tensor(f"mt{p}", [D, D], DT_X, kind="ExternalInput").ap()
             for p in range(3)] if need_mt else None)
    emb_d = nc.dram_tensor("wanom_emb", [128, ND, BC, BC], bf16,
                           kind="ExternalInput").ap()
    w1_d = nc.dram_tensor("w_f1", [NC3, 128, D], bf16, kind="ExternalInput").ap()
    w2_d = nc.dram_tensor("w_f2", [ND, 128, D], bf16, kind="ExternalInput").ap()
    b1_d = nc.dram_tensor("b_f1", [1, D], bf16, kind="ExternalInput").ap()
    b2_d = nc.dram_tensor("b_f2", [1, D], bf16, kind="ExternalInput").ap()
    lng_d = lnb_d = None
    if not ln_trivial:
        lng_d = nc.dram_tensor("ln_g", [BC, D], f32, kind="ExternalInput").ap()
        lnb_d = nc.dram_tensor("ln_b", [BC, D], f32, kind="ExternalInput").ap()
    out_d = nc.dram_tensor("out", [BC, D], f32, kind="ExternalOutput").ap()

    with tile.TileContext(nc) as tc, ExitStack() as ctx:
        const_p = ctx.enter_context(tc.tile_pool(name="const", bufs=1))
        main_p = ctx.enter_context(tc.tile_pool(name="main", bufs=1))
        sm_p = ctx.enter_context(tc.tile_pool(name="small", bufs=2))
        tmp_p = ctx.enter_context(tc.tile_pool(name="tmp", bufs=2))
        ps_big = ctx.enter_context(tc.tile_pool(name="psb", bufs=5, space="PSUM"))
        ps_med = ctx.enter_context(tc.tile_pool(name="psm", bufs=1, space="PSUM"))
        ps_sm = ctx.enter_context(tc.tile_pool(name="pss", bufs=2, space="PSUM"))
        tailA_p = (ctx.enter_context(tc.tile_pool(name="tailA", bufs=1))
                   if early_tail else None)
        es2 = ExitStack()   # E matrices; closed after last r
        e_p = es2.enter_context(tc.tile_pool(name="emat", bufs=1))
        es1 = ExitStack()   # W matrices + projT; closed after last scores
        w_p = es1.enter_context(tc.tile_pool(name="w", bufs=1))
        proj_p = es1.enter_context(tc.tile_pool(name="proj", bufs=1))

        # ---- constants ----
        ones_row = const_p.tile([1, 128], bf16)
        nc.vector.memset(ones_row[:], 1.0)
        ones4 = const_p.tile([1, BC], bf16)
        nc.vector.memset(ones4[:], 1.0)
        eyerows = const_p.tile([1, BC, BC], bf16)
        nc.vector.memset(eyerows[:], 0.0)
        for s in range(BC):
            nc.vector.memset(eyerows[:, s, s : s + 1], 1.0)
        iot_t = const_p.tile([128, 128], mybir.dt.int32)
        nc.gpsimd.iota(iot_t[:], pattern=[[1, 128]], base=0, channel_multiplier=-1)
        ident_f = const_p.tile([128, 128], f32)
        nc.vector.tensor_scalar(ident_f[:], iot_t[:], scalar1=0, scalar2=None,
                                op0=ALU.is_equal)
        ident_b = const_p.tile([128, 128], bf16)
        nc.vector.tensor_copy(ident_b[:], ident_f[:])
        warm_t = const_p.tile([1, 1], f32)
        nc.scalar.sqrt(warm_t[:], ones4[0:1, 0:1])
        # HAM warm-up: dep-free matmuls keep the PE busy from t=0 so the
        # clock gate opens (1.2 -> 2.4 GHz) before the DMA-fed work lands.
        warm_ps = ps_med.tile([128, 64], f32, tag="pm", name="warm_ps")
        for _ in range(90):
            nc.tensor.matmul(warm_ps[:], lhsT=ident_b[:],
                             rhs=ident_b[0:128, 0:64], start=True, stop=True)

        emb_t = const_p.tile([128, ND, BC, BC], bf16)
        nc.scalar.dma_start(emb_t[:], emb_d[:])
        fb_t = const_p.tile([1, BC, L], bf16)
        ob_t = const_p.tile([1, BC, L], bf16)
        for s in range(BC):
            nc.scalar.dma_start(fb_t[:, s, :], fb_d[s : s + 1, :])
            nc.scalar.dma_start(ob_t[:, s, :], ob_d[s : s + 1, :])
        b1_t = const_p.tile([1, D], bf16)
        nc.scalar.dma_start(b1_t[:], b1_d[:])
        b2_t = const_p.tile([1, D], bf16)
        nc.scalar.dma_start(b2_t[:], b2_d[:])
        lng_t = lnb_t = None
        if not ln_trivial:
            lng_t = const_p.tile([BC, D], f32)
            nc.scalar.dma_start(lng_t[:], lng_d[:])
            lnb_t = const_p.tile([BC, D], f32)
            nc.scalar.dma_start(lnb_t[:], lnb_d[:])

        # ---- big inputs ----
        xq_t = main_p.tile([128, ND, SQ], DT_X)
        for k in range(ND):
            nc.sync.dma_start(xq_t[:, k, :], xq_d[k])
        xo_t = None
        if SO:
            xo_t = main_p.tile([128, ND, SO], DT_SC)
            for k in range(ND):
                nc.sync.dma_start(xo_t[:, k, :], xo_d[k])
        xo8_t = xo_t
        if SO and need_xo8:
            xo8_t = main_p.tile([128, ND, SO], DT_X)
            for k in range(ND):
                nc.sync.dma_start(xo8_t[:, k, :], xo8_d[k])
        w_ts = []
        for p in range(3):
            wt = w_p.tile([128, ND, D], DT_X, name=f"w{p}")
            nc.gpsimd.dma_start(
                wt[:, 0 : ND // 2, :],
                m_d[p][0 : D // 2].rearrange("(k p) c -> p k c", p=128))
            nc.gpsimd.dma_start(
                wt[:, ND // 2 :, :],
                m_d[p][D // 2 :].rearrange("(k p) c -> p k c", p=128))
            w_ts.append(wt)
        wt_ts = []
        if need_mt:
            for p in range(3):
                wtt = w_p.tile([128, ND, D], DT_X, name=f"wt{p}")
                nc.gpsimd.dma_start(
                    wtt[:], mt_d[p].rearrange("(k p) c -> p k c", p=128)
                )
                wt_ts.append(wtt)
        x_t = main_p.tile([128, BC, NL, D], bf16)
        for s in range(BC):
            nc.sync.dma_start(
                x_t[:, s], x_d[s].rearrange("(t p) d -> p t d", p=128)
            )
        xqb_t = xq_t
        if need_xqb:
            xqb_t = w_p.tile([128, ND, SQ], bf16, name="xqb")
            for k in range(ND):
                nc.sync.dma_start(xqb_t[:, k, :], xqb_d[k])

        def _load_w1_half(pool, half, name):
            w1h = pool.tile([128, NC3, 512], bf16, name=name)
            hs = slice(half * 512, (half + 1) * 512)
            for t in range(3):
                nc.gpsimd.dma_start(
                    w1h[:, t * ND : (t + 1) * ND, :],
                    w1_d[t * ND : (t + 1) * ND, :, hs].rearrange(
                        "k p n -> p k n"),
                )
            return w1h

        w1h_t = [None, None]
        if early_tail:
            w1h_t[0] = _load_w1_half(tailA_p, 0, "w1a")

        G_all = main_p.tile([128, NL, BC, 12], bf16)   # col = t*4 + s
        nc.vector.memset(G_all[:], 0.0)
        gcol = main_p.tile([128, maxF, BC], f32)

        # ---- projections: chains grouped by (p, m) so consecutive matmuls
        # share the stationary weight block (LDWEIGHTS hides under streams);
        # both pairs interleave when SBUF allows.
        DR = mybir.MatmulPerfMode.DoubleRow if USE_FP8 else None
        NK = ND // 2 if USE_FP8 else ND

        def alloc_projT(pr):
            tag_pr = pr if interleave_proj else 0
            return [
                proj_p.tile([128, ND, 2, geos[pr]["w"]], DT_SC,
                            tag=f"pj{p}_{tag_pr}", name=f"pj{p}_{pr}")
                for p in range(3)
            ]

        def emit_proj(prs, projTs):
            units = []
            for pr in prs:
                g = geos[pr]
                if not g["have"]:
                    continue
                poff = offq if g["side_q"] else offo
                assert poff[2 * pr + 1] == poff[2 * pr] + g["w"]
                if 2 * g["w"] <= 512:
                    units.append((pr, None))
                else:
                    units.append((pr, 0))
                    units.append((pr, 1))
            for p in range(3):
                for m in range(ND):
                    msl = slice(m * 128, (m + 1) * 128)
                    chains = []
                    for pr, sp in units:
                        w = geos[pr]["w"]
                        shape = [128, 2, w] if sp is None else [128, w]
                        ps = ps_big.tile(shape, f32, tag="ps",
                                         name=f"pjps{pr}")
                        chains.append((pr, sp, ps))
                    for k in range(NK):
                        for pr, sp, ps in chains:
                            g = geos[pr]
                            xsrc = xq_t if g["side_q"] else xo8_t
                            poff = offq if g["side_q"] else offo
                            wm = (w_ts if g["side_q"] else wt_ts)[p]
                            off = poff[2 * pr + (sp or 0)]
                            wid = 2 * g["w"] if sp is None else g["w"]
                            if USE_FP8:
                                nc.tensor.matmul(
                                    ps[:],
                                    lhsT=wm[:, 2 * k : 2 * k + 2, msl],
                                    rhs=xsrc[:, 2 * k : 2 * k + 2,
                                             off : off + wid],
                                    start=(k == 0), stop=(k == NK - 1),
                                    perf_mode=DR,
                                )
                            else:
                                nc.tensor.matmul(
                                    ps[:], lhsT=wm[:, k, msl],
                                    rhs=xsrc[:, k, off : off + wid],
                                    start=(k == 0), stop=(k == NK - 1),
                                )
                    for i, (pr, sp, ps) in enumerate(chains):
                        dst = (projTs[pr][p][:, m] if sp is None
                               else projTs[pr][p][:, m, sp])
                        if (m + i) % 2 == 0:
                            nc.vector.tensor_copy(dst, ps[:])
                        else:
                            nc.scalar.activation(dst, ps[:], AF.Copy)

        projTs = {}
        if interleave_proj:
            for pr in range(2):
                if geos[pr]["have"]:
                    projTs[pr] = alloc_projT(pr)
            emit_proj([0, 1], projTs)

        # ---- gates (all samples): logits in ONE [4, wg] PSUM ----
        ga_ps = ps_med.tile([BC, wg], f32, tag="pm")
        for k in range(ND):
            for s in range(BC):
                nc.tensor.matmul(
                    ga_ps[:, 0 : CQs[s]], lhsT=emb_t[:, k, s, :],
                    rhs=xqb_t[:, k, offq[s] : offq[s] + CQs[s]],
                    start=(k == 0 and s == 0), stop=False,
                )
        for s in range(BC):
            nc.tensor.matmul(
                ga_ps[:], lhsT=eyerows[:, s, :], rhs=fb_t[:, s, 0:wg],
                start=False, stop=(s == BC - 1),
            )
        grow = sm_p.tile([BC, wg], f32, tag="grow", bufs=1)
        gs_t = sm_p.tile([BC, 1], f32, tag="gs", bufs=1)
        nc.scalar.activation(grow[:], ga_ps[:], AF.Exp, scale=1.0 / PRE,
                             accum_out=gs_t[:])
        nc.vector.tensor_scalar_max(gs_t[:], gs_t[:], 1e-8)
        rg_t = sm_p.tile([BC, 1], f32, tag="rg", bufs=1)
        nc.vector.reciprocal(rg_t[:], gs_t[:])
        nc.vector.tensor_scalar_mul(grow[:], grow[:], rg_t[:])
        for it in range(maxF):
            gt_ps = ps_med.tile([128, BC], f32, tag="pm")
            nc.tensor.transpose(gt_ps[:], grow[:, it * 128 : (it + 1) * 128],
                                ident_f[0:BC, 0:BC])
            nc.vector.tensor_copy(gcol[:, it, :], gt_ps[:])
            for s in range(BC):
                if it < geos[s // 2]["F"]:
                    nc.vector.tensor_copy(G_all[:, it, s, s : s + 1],
                                          gt_ps[:, s : s + 1])

        # ---- per pair: (projections) -> scores -> E -> coeffs -> r ----
        for pr in range(2):
            g = geos[pr]
            F, OJ, NO, w, lo = g["F"], g["OJ"], g["NO"], g["w"], g["lo"]
            if not g["have"]:
                continue
            s0, s1 = 2 * pr, 2 * pr + 1
            if not interleave_proj:
                projTs[pr] = alloc_projT(pr)
                emit_proj([pr], projTs)
            projT = projTs[pr]

            for sp in range(2):
                s4 = s0 + sp
                E_sup = e_p.tile([128, F, NO], bf16, tag=f"Es{sp}",
                                 name=f"Es{sp}_{pr}")
                E_rep = e_p.tile([128, F, NO], bf16, tag=f"Er{sp}",
                                 name=f"Er{sp}_{pr}")
                co_sup = sm_p.tile([128, F], bf16, tag=f"cos{sp}", bufs=1,
                                   name=f"cos{sp}_{pr}")
                co_rep = sm_p.tile([128, F], bf16, tag=f"cor{sp}", bufs=1,
                                   name=f"cor{sp}_{pr}")
                sc_dr = USE_FP8 and FP8_SCORES
                NKS = ND // 2 if sc_dr else ND
                for it in range(F):
                    isl = slice(it * 128, (it + 1) * 128)
                    ps3 = {}
                    for p in (0, 2, 1):   # sup, rep, con
                        ps = ps_big.tile([128, NO], f32, tag="ps",
                                         name=f"sc{p}")
                        ps3[p] = ps
                        for k in range(NKS):
                            if sc_dr:
                                ksl = slice(2 * k, 2 * k + 2)
                                if g["side_q"]:
                                    lhsT = projT[p][:, ksl, sp, isl]
                                    rhs = xo_t[:, ksl,
                                               offo[s4] : offo[s4] + NO]
                                else:
                                    lhsT = xq_t[:, ksl,
                                                offq[s4] + it * 128 :
                                                offq[s4] + (it + 1) * 128]
                                    rhs = projT[p][:, ksl, sp, 0:NO]
                                nc.tensor.matmul(
                                    ps[:], lhsT=lhsT, rhs=rhs,
                                    start=(k == 0),
                                    stop=(k == NKS - 1 and p == 1),
                                    perf_mode=mybir.MatmulPerfMode.DoubleRow,
                                )
                            else:
                                if g["side_q"]:
                                    lhsT = projT[p][:, k, sp, isl]
                                    rhs = xo_t[:, k, offo[s4] : offo[s4] + NO]
                                else:
                                    lhsT = xqb_t[:, k,
                                                 offq[s4] + it * 128 :
                                                 offq[s4] + (it + 1) * 128]
                                    rhs = projT[p][:, k, sp, 0:NO]
                                nc.tensor.matmul(ps[:], lhsT=lhsT, rhs=rhs,
                                                 start=(k == 0),
                                                 stop=(k == NKS - 1 and p == 1))
                        if p != 1:
                            # option-mask bias row closes the sup/rep groups
                            nc.tensor.matmul(ps[:], lhsT=ones_row[:],
                                             rhs=ob_t[:, s4, OJ:L],
                                             start=False, stop=True)
                    ps_sup, ps_rep, ps_con = ps3[0], ps3[2], ps3[1]

                    T_t = tmp_p.tile([128, NO], f32, tag="T")
                    nc.scalar.activation(T_t[:], ps_con[:], AF.Tanh,
                                         scale=SCALE / PRE)
                    A_t = tmp_p.tile([128, NO], f32, tag="A")
                    nc.vector.scalar_tensor_tensor(
                        A_t[:], in0=ps_rep[:], scalar=SCALE / PRE, in1=T_t[:],
                        op0=ALU.mult, op1=ALU.add,
                    )
                    rs_sup = sm_p.tile([128, 1], f32, tag="rss")
                    nc.scalar.activation(E_sup[:, it], ps_sup[:], AF.Exp,
                                         scale=SCALE / PRE, accum_out=rs_sup[:])
                    rs_rep = sm_p.tile([128, 1], f32, tag="rsr")
                    nc.scalar.activation(E_rep[:, it], A_t[:], AF.Exp,
                                         accum_out=rs_rep[:])
                    rc_sup = sm_p.tile([128, 1], f32, tag="rcs")
                    nc.vector.reciprocal(rc_sup[:], rs_sup[:])
                    nc.vector.tensor_mul(co_sup[:, it : it + 1],
                                         gcol[:, it, s4 : s4 + 1], rc_sup[:])
                    rc_rep = sm_p.tile([128, 1], f32, tag="rcr")
                    nc.vector.reciprocal(rc_rep[:], rs_rep[:])
                    nc.vector.tensor_mul(co_rep[:, it : it + 1],
                                         gcol[:, it, s4 : s4 + 1], rc_rep[:])

                # r vectors: G col 4+s (rep), 8+s (sup)
                for t, (E_t, co_t) in enumerate(((E_rep, co_rep),
                                                 (E_sup, co_sup))):
                    for jt in range(NO // 128):
                        jsl = slice(jt * 128, (jt + 1) * 128)
                        r_ps = ps_sm.tile([128, 1], f32, tag="r")
                        for it in range(F):
                            nc.tensor.matmul(
                                r_ps[:], lhsT=E_t[:, it, jsl],
                                rhs=co_t[:, it : it + 1],
                                start=(it == 0), stop=(it == F - 1),
                            )
                        nc.vector.tensor_copy(
                            G_all[:, g["J0"] + jt, s4,
                                  4 * (t + 1) + s4 : 4 * (t + 1) + s4 + 1],
                            r_ps[:],
                        )

        es1.close()
        es2.close()

        # ---- remaining tail weights (after proj/W pools freed) ----
        tail_p = ctx.enter_context(tc.tile_pool(name="tail", bufs=1))
        if not early_tail:
            w1h_t[0] = _load_w1_half(tail_p, 0, "w1a2")
        w1h_t[1] = _load_w1_half(tail_p, 1, "w1b")
        w2_t = tail_p.tile([128, ND, D], bf16, name="w2sb")
        nc.gpsimd.dma_start(w2_t[:], w2_d.rearrange("k p n -> p k n"))

        # ---- pool: pooled[t*4+s, :] = sum_l G[l, t*4+s] * x_s[l, :] ----
        seq = []
        for s4 in range(BC):
            g = geos[s4 // 2]
            rts = sorted(set(range(g["F"]))
                         | (set(range(g["J0"], NL)) if g["have"] else set()))
            seq.extend((s4, rt) for rt in rts)
        pooled_sb = main_p.tile([12, D], bf16)
        fused_sb = main_p.tile([128, ND, 3, BC], bf16)
        for half in range(2):
            hs = slice(half * 512, (half + 1) * 512)
            po_ps = ps_med.tile([12, 512], f32, tag="pm")
            for i, (s4, rt) in enumerate(seq):
                nc.tensor.matmul(
                    po_ps[:], lhsT=G_all[:, rt, s4, :], rhs=x_t[:, s4, rt, hs],
                    start=(i == 0), stop=(i == len(seq) - 1),
                )
            nc.vector.tensor_copy(pooled_sb[:, hs], po_ps[:])
            for m in range(half * 4, half * 4 + 4):
                tr_ps = ps_sm.tile([128, 12], bf16, tag="r")
                nc.tensor.transpose(tr_ps[:],
                                    pooled_sb[:, m * 128 : (m + 1) * 128],
                                    ident_b[0:12, 0:12])
                nc.vector.tensor_copy(fused_sb[:, m], tr_ps[:])

        # ---- MLP tail + LayerNorm, all in [4, 1024] row form ----
        h_sb = main_p.tile([BC, D], bf16)
        hT_sb = main_p.tile([128, ND, BC], bf16)
        for half in range(2):
            hs = slice(half * 512, (half + 1) * 512)
            h_ps = ps_big.tile([BC, 512], f32, tag="ps")
            for t in range(3):
                for m in range(ND):
                    nc.tensor.matmul(
                        h_ps[:], lhsT=fused_sb[:, m, t, :],
                        rhs=w1h_t[half][:, t * ND + m, :],
                        start=(t == 0 and m == 0), stop=False,
                    )
            nc.tensor.matmul(h_ps[:], lhsT=ones4[:], rhs=b1_t[:, hs],
                             start=False, stop=True)
            nc.scalar.activation(h_sb[:, hs], h_ps[:], AF.Relu)
        for m in range(ND):
            ht_ps = ps_sm.tile([128, BC], bf16, tag="r")
            nc.tensor.transpose(ht_ps[:], h_sb[:, m * 128 : (m + 1) * 128],
                                ident_b[0:BC, 0:BC])
            nc.vector.tensor_copy(hT_sb[:, m], ht_ps[:])

        y_sb = main_p.tile([BC, D], f32)
        s1_t = sm_p.tile([BC, 2], f32, tag="s1", bufs=1)
        s2_t = sm_p.tile([BC, 2], f32, tag="s2", bufs=1)
        for half in range(2):
            hs = slice(half * 512, (half + 1) * 512)
            y_ps = ps_big.tile([BC, 512], f32, tag="ps")
            for k in range(ND):
                nc.tensor.matmul(y_ps[:], lhsT=hT_sb[:, k, :],
                                 rhs=w2_t[:, k, hs],
                                 start=(k == 0), stop=False)
            nc.tensor.matmul(y_ps[:], lhsT=ones4[:], rhs=b2_t[:, hs],
                             start=False, stop=True)
            if half == 0:
                nc.scalar.activation(y_sb[:, hs], y_ps[:], AF.Copy,
                                     accum_out=s1_t[:, half : half + 1])
            else:
                # split across engines: DVE copies+sums while ACT squares
                nc.vector.tensor_copy(y_sb[:, hs], y_ps[:])
                nc.vector.tensor_reduce(s1_t[:, half : half + 1],
                                        y_ps[:], axis=mybir.AxisListType.X,
                                        op=ALU.add)
            sq = tmp_p.tile([BC, 512], f32, tag="sq", bufs=2)
            nc.scalar.activation(sq[:], y_ps[:], AF.Square,
                                 accum_out=s2_t[:, half : half + 1])

        mu_t = sm_p.tile([BC, 1], f32, tag="mu", bufs=1)
        nc.vector.tensor_reduce(mu_t[:], s1_t[:], axis=mybir.AxisListType.X,
                                op=ALU.add)
        nc.scalar.mul(mu_t[:], mu_t[:], 1.0 / D)
        msq_t = sm_p.tile([BC, 1], f32, tag="msq", bufs=1)
        nc.vector.tensor_reduce(msq_t[:], s2_t[:], axis=mybir.AxisListType.X,
                                op=ALU.add)
        nc.scalar.mul(msq_t[:], msq_t[:], 1.0 / D)
        m2_t = sm_p.tile([BC, 1], f32, tag="m2", bufs=1)
        nc.vector.tensor_mul(m2_t[:], mu_t[:], mu_t[:])
        var_t = sm_p.tile([BC, 1], f32, tag="var", bufs=1)
        nc.vector.tensor_scalar(var_t[:], msq_t[:], scalar1=m2_t[:],
                                scalar2=LN_EPS, op0=ALU.subtract, op1=ALU.add)
        sd_t = sm_p.tile([BC, 1], f32, tag="sd", bufs=1)
        nc.scalar.sqrt(sd_t[:], var_t[:])
        rstd_t = sm_p.tile([BC, 1], f32, tag="rstd", bufs=1)
        nc.vector.reciprocal(rstd_t[:], sd_t[:])
        nmr_t = sm_p.tile([BC, 1], f32, tag="nmr", bufs=1)
        nc.vector.tensor_scalar(nmr_t[:], mu_t[:], scalar1=rstd_t[:],
                                scalar2=-1.0, op0=ALU.mult, op1=ALU.mult)

        z_sb = main_p.tile([BC, D], f32)
        for half in range(2):
            hs = slice(half * 512, (half + 1) * 512)
            if half == 0:
                nc.scalar.activation(z_sb[:, hs], y_sb[:, hs], AF.Identity,
                                     scale=rstd_t[:], bias=nmr_t[:])
            else:
                # other engine: runs concurrently with half 0 + its DMA
                nc.vector.tensor_scalar(z_sb[:, hs], y_sb[:, hs],
                                        scalar1=mu_t[:], scalar2=rstd_t[:],
                                        op0=ALU.subtract, op1=ALU.mult)
            if not ln_trivial:
                nc.vector.tensor_mul(z_sb[:, hs], z_sb[:, hs], lng_t[:, hs])
                nc.vector.tensor_add(z_sb[:, hs], z_sb[:, hs], lnb_t[:, hs])
            nc.sync.dma_start(out_d[:, hs], z_sb[:, hs])

    nc.compile()
    return nc


def _masks(x_ids, pad_idx, sep_idx):
    valid = x_ids != pad_idx
    sepm = x_ids == sep_idx
    has = sepm.any(axis=1)
    first = sepm.argmax(axis=1)
    vlen = valid.sum(axis=1)
    fb = np.clip(vlen // 2, 1, max(1, L - 2))
    sp = np.where(has, first, fb)
    pos = np.arange(L)
    fmask = (pos[None, :] < sp[:, None]) & valid
    omask = (pos[None, :] > sp[:, None]) & valid
    return sp, fmask, omask


def _host_prep_fast(inputs):
    import os

    x = np.asarray(inputs["x"], dtype=np.float32)
    x_ids = np.asarray(inputs["x_ids"])
    pad_idx = int(np.asarray(inputs["pad_idx"]))
    sep_idx = int(np.asarray(inputs["sep_idx"]))
    assert x.shape == (B, L, D), x.shape
    np_x = np_fp8 if USE_FP8 else np_bf16

    sp, fmask, omask = _masks(x_ids, pad_idx, sep_idx)
    fb = np.where(fmask, 0.0, FBIAS * PRE).astype(np_bf16)
    ob = np.where(omask, 0.0, OBIAS_RAW * PRE).astype(np_bf16)

    order = np.argsort(-sp, kind="stable")
    F_all = np.maximum(np.ceil(sp / 128).astype(int), 1)
    J0_all = np.minimum((sp + 1) // 128, NL)
    pair_geo = tuple(
        (int(F_all[order[pr * 16 : (pr + 1) * 16]].max()),
         int(J0_all[order[pr * 16 : (pr + 1) * 16]].min()))
        for pr in range(2)
    )
    fbnd = os.environ.get("FORCE_BOUNDS")
    if fbnd:
        f0, j0, f1, j1 = (int(v) for v in fbnd.split(","))
        pair_geo = ((f0, j0), (f1, j1))
    geos = [_geo(F, J0) for (F, J0) in pair_geo]
    need_mt = any(g["have"] and not g["side_q"] for g in geos)
    np_sc = np_fp8 if (USE_FP8 and FP8_SCORES) else np_bf16
    has_kside = any(g["have"] and not g["side_q"] for g in geos)
    need_xqb = USE_FP8
    need_xo8 = (np_sc != np_x) and has_kside
    CQs = [geos[s // 2]["CQ"] for s in range(BC)]
    NOs = [geos[s // 2]["NO"] for s in range(BC)]
    OJs = [geos[s // 2]["OJ"] for s in range(BC)]
    SQ, SO = sum(CQs), sum(NOs)

    def w(name):
        return np.ascontiguousarray(np.asarray(inputs[name], dtype=np.float32))

    shared = {}
    for p, (qn, kn) in enumerate((("w_sq", "w_sk"), ("w_cq", "w_ck"),
                                  ("w_rq", "w_rk"))):
        shared[f"m{p}"] = _m_matrix(inputs[qn], inputs[kn])
        if need_mt:
            shared[f"mt{p}"] = _m_matrix(inputs[qn], inputs[kn], transposed=True)

    wanom_pm = w("w_anom").reshape(ND, 128).T            # [128, ND]
    emb = np.zeros((128, ND, BC, BC), np.float32)
    for s in range(BC):
        emb[:, :, s, s] = wanom_pm * PRE
    shared["wanom_emb"] = emb.astype(np_bf16)

    shared["w_f1"] = np.ascontiguousarray(
        w("w_f1").reshape(NC3, 128, D)).astype(np_bf16)
    shared["w_f2"] = np.ascontiguousarray(
        w("w_f2").reshape(ND, 128, D)).astype(np_bf16)
    shared["b_f1"] = w("b_f1").reshape(1, D).astype(np_bf16)
    shared["b_f2"] = w("b_f2").reshape(1, D).astype(np_bf16)
    ln_g, ln_b = w("ln_g"), w("ln_b")
    ln_trivial = bool(np.all(ln_g == 1.0) and np.all(ln_b == 0.0))
    if not ln_trivial:
        shared["ln_g"] = np.broadcast_to(ln_g.reshape(1, D), (BC, D)).copy()
        shared["ln_b"] = np.broadcast_to(ln_b.reshape(1, D), (BC, D)).copy()

    in_maps = []
    core_idx = []
    for c in range(NCORES):
        idx = order[np.arange(BC) * NCORES + c]
        core_idx.append(idx)
        xs = x[idx]                                      # [BC, L, D] f32
        m = dict(shared)
        m["x"] = xs.astype(np_bf16)
        xsT = np.ascontiguousarray(xs.transpose(2, 0, 1))   # [D, BC, L] f32
        xq_f = np.ascontiguousarray(np.concatenate(
            [xsT[:, s, 0 : CQs[s]] for s in range(BC)], axis=1,
        ))
        m["xq"] = xq_f.reshape(ND, 128, SQ).astype(np_x)
        if need_xqb:
            m["xqb"] = xq_f.reshape(ND, 128, SQ).astype(np_bf16)
        if SO:
            xo_f = np.ascontiguousarray(np.concatenate(
                [xsT[:, s, OJs[s] : L] for s in range(BC)], axis=1,
            ))
            m["xo"] = xo_f.reshape(ND, 128, SO).astype(np_sc)
            if need_xo8:
                m["xo8"] = xo_f.reshape(ND, 128, SO).astype(np_x)
        m["fbias"] = np.ascontiguousarray(fb[idx])
        m["obias"] = np.ascontiguousarray(ob[idx])
        in_maps.append(m)
    return in_maps, (pair_geo, ln_trivial), core_idx


def get_program_fast(key):
    if key not in _PROGRAM_CACHE:
        pair_geo, ln_trivial = key
        _PROGRAM_CACHE[key] = build_program_fast(pair_geo, ln_trivial)
    return _PROGRAM_CACHE[key]


def run(trace=False, **inputs):
    use_m = all(
        not np.any(np.asarray(inputs[n]))
        for n in ("b_sq", "b_sk", "b_cq", "b_ck", "b_rq", "b_rk")
    )
    if not use_m:
        return _run_legacy(trace=trace, **inputs)
    in_maps, key, core_idx = _host_prep_fast(inputs)
    nc = get_program_fast(key)
    res = bass_utils.run_bass_kernel_spmd(
        nc, in_maps, core_ids=list(range(NCORES)), trace=trace
    )
    out = np.empty((B, D), np.float32)
    for c in range(NCORES):
        out[core_idx[c]] = res.results[c]["out"]
    return out, res


def kernel(**inputs):
    out, _ = run(trace=False, **inputs)
    return out


# ---------------------------------------------------------------------------
# Fallback (nonzero projection biases): exact numpy reference. This path is
# not expected in practice (setup_inputs uses zero biases); correctness over
# speed.
# ---------------------------------------------------------------------------

NEG = -9.0e15


def _run_legacy(trace=False, **inputs):
    x = np.asarray(inputs["x"], dtype=np.float32)
    x_ids = np.asarray(inputs["x_ids"])
    pad_idx = int(np.asarray(inputs["pad_idx"]))
    sep_idx = int(np.asarray(inputs["sep_idx"]))

    def w(name):
        return np.asarray(inputs[name], dtype=np.float32)

    _, fmask, omask = _masks(x_ids, pad_idx, sep_idx)

    al = (x @ w("w_anom") + w("b_anom"))[..., 0]
    al = np.where(fmask, al, NEG)
    al -= al.max(axis=1, keepdims=True)
    gate = np.exp(al)
    gate /= gate.sum(axis=1, keepdims=True)
    gate = gate * fmask
    gate = gate / np.clip(gate.sum(axis=1, keepdims=True), 1e-8, None)

    scale = 1.0 / np.sqrt(D)
    pair = fmask[:, :, None] & omask[:, None, :]

    def attn(sq, bq, sk, bk, extra=None):
        q = x @ w(sq) + w(bq)
        k = x @ w(sk) + w(bk)
        s = np.einsum("bid,bjd->bij", q, k) * scale
        if extra is not None:
            s = s + extra
        s = np.where(pair, s, NEG)
        s -= s.max(axis=2, keepdims=True)
        e = np.exp(s)
        return e / e.sum(axis=2, keepdims=True)

    qc = x @ w("w_cq") + w("b_cq")
    kc = x @ w("w_ck") + w("b_ck")
    conf = np.tanh(np.einsum("bid,bjd->bij", qc, kc) * scale)
    sup_a = attn("w_sq", "b_sq", "w_sk", "b_sk")
    rep_a = attn("w_rq", "b_rq", "w_rk", "b_rk", extra=conf)

    rep_vec = np.einsum("bij,bjd->bid", rep_a, x)
    sup_vec = np.einsum("bij,bjd->bid", sup_a, x)
    fused = np.concatenate([
        np.einsum("bl,bld->bd", gate, x),
        np.einsum("bl,bld->bd", gate, rep_vec),
        np.einsum("bl,bld->bd", gate, sup_vec),
    ], axis=-1)
    fused = np.maximum(fused @ w("w_f1") + w("b_f1"), 0.0) @ w("w_f2") + w("b_f2")
    mu = fused.mean(axis=-1, keepdims=True)
    var = fused.var(axis=-1, keepdims=True)
    fused = (fused - mu) / np.sqrt(var + LN_EPS) * w("ln_g") + w("ln_b")

    class _Res:
        results = None
        exec_time_ns = None

    return fused.astype(np.float32), _Res()


# revision 35
# speedup vs baseline: 1.0221x; 1.0221x over previous
"""Trainium2 Bass kernel for nn_BertCounterFactTransformer.

Contract: kernel(**inputs) takes FULL unsharded numpy inputs (as produced by
reference.setup_inputs()) and returns the FULL [32, 1024] float32 output.

Strategy (data-parallel over batch, 8 cores x 4 samples):
  - Host: compute sep positions from x_ids, SORT samples by sep position and
    assign sorted rank r -> core (r % 8), slot (r // 8) so the per-slot-pair
    tile bounds are tight and uniform across cores. Precompute
    M_p = W_pq @ W_pk^T, pre-scaled x64 into fp8 e4m3 normal range, so
    scores are x M x^T (only the smaller side is ever projected). x is
    packed per-core into fp8 xq (false-row cols) / xo (option cols) plus a
    bf16 xqb copy for the precision-critical gate.
  - Device, per pair of slots (F tiles of false rows, option cols from OJ):
      gate     all-4-sample anomaly logits in ONE [4, wg] PSUM (bf16) via
               block-diagonal embedded w_anom; false-mask folded as bias
               rows; exp+normalize row-wise; PE-transpose to columns
      proj     qT = (x M_p)^T (or M_p x_opt^T if the option side is
               smaller) as fp8 DoubleRow chains, 2 k-tiles per matmul and
               2 samples per rhs (width<=512)
      scores   S = q x_opt^T as fp8 DoubleRow chains; option mask via
               bias rows into PSUM; exp/tanh scales fold the 1/64
      E_sup = exp(S_sup/32 + ob), E_rep = exp(S_rep/32 + tanh(S_con/32) + ob)
      coeff_t = gate / rowsum(E_t);  r_t = E_t^T @ coeff_t  (width-1 chains)
  - Pool: ONE [12, 512] PSUM accumulates x_s^T @ [gate|r_rep|r_sup] for all
    4 samples via 12-col G with per-sample zero blocks; PE-transpose the
    [12, 1024] result into fused^T columns.
  - Tail in row form: h = relu(fused @ W1 + b1), y = h @ W2 + b2, LayerNorm
    along the free dim (skipping the affine when ln_g==1, ln_b==0), split
    [4, 512] output DMAs. W1's first half is prefetched early when SBUF
    allows so the tail never waits on HBM.

Key identity: gate @ (attn @ x) == (gate @ attn) @ x, so [L,D] attention
outputs are never materialized. Nonzero projection biases fall back to an
exact numpy path (not expected in practice).
"""

import sys

if "/opt/trn_rl_repo" not in sys.path:
    sys.path.insert(0, "/opt/trn_rl_repo")

import numpy as np
import ml_dtypes
from contextlib import ExitStack

np_bf16 = ml_dtypes.bfloat16
np_fp8 = ml_dtypes.float8_e4m3

import concourse.bacc as bacc
import concourse.bass as bass
import concourse.mybir as mybir
import concourse.tile as tile
from concourse import bass_utils

f32 = mybir.dt.float32
bf16 = mybir.dt.bfloat16
fp8 = mybir.dt.float8e4
AF = mybir.ActivationFunctionType
ALU = mybir.AluOpType

B, L, D = 32, 512, 1024
NCORES = 8
BC = B // NCORES          # samples per core
NL = L // 128             # 4 L-tiles
ND = D // 128             # 8 D-tiles
NC3 = 3 * D // 128        # 24 tiles of the 3D fused dim
SCALE = 1.0 / 32.0        # 1/sqrt(D)
OBIAS_RAW = -960.0        # -30 after * SCALE
FBIAS = -30.0
LN_EPS = 1e-5
USE_FP8 = True            # fp8 e4m3 + DoubleRow for the projection GEMMs
FP8_SCORES = True         # score operands (projT, xo) also fp8 + DoubleRow
PRE = 64.0 if USE_FP8 else 1.0   # pre-scale on M/w_anom (fp8 normal range)

_PROGRAM_CACHE = {}
_M_CACHE = {}


def _m_matrix(wq, wk, transposed=False):
    import hashlib
    wq = np.asarray(wq, dtype=np.float32)
    wk = np.asarray(wk, dtype=np.float32)
    key = (hashlib.blake2b(wq.tobytes() + wk.tobytes(), digest_size=16).digest(),
           transposed, USE_FP8)
    if key not in _M_CACHE:
        m = wq @ wk.T
        if transposed:
            m = m.T
        m = np.ascontiguousarray(m)
        if USE_FP8:
            _M_CACHE[key] = np.clip(m * PRE, -240, 240).astype(np_fp8)
        else:
            _M_CACHE[key] = m.astype(np_bf16)
    return _M_CACHE[key]


def _geo(F, J0):
    OJ = J0 * 128
    NO = L - OJ
    CQ = F * 128
    have = NO > 0
    side_q = (CQ <= NO) if have else True
    w = (CQ if side_q else NO) if have else 0
    lo = 0 if side_q else OJ
    return dict(F=F, J0=J0, OJ=OJ, NO=NO, CQ=CQ, have=have,
                side_q=side_q, w=w, lo=lo)


def build_program_fast(pair_geo, ln_trivial=False):
    """pair_geo = ((F0, J0_0), (F1, J0_1)); pair p covers slots {2p, 2p+1}.
    Computing a superset is always correct (bias masks zero it)."""
    nc = bacc.Bacc(
        "TRN2",
        target_bir_lowering=False,
        debug=False,
        enable_asserts=False,
        num_devices=NCORES,
    )

    geos = [_geo(F, J0) for (F, J0) in pair_geo]
    need_mt = any(g["have"] and not g["side_q"] for g in geos)
    wg = max(max(g["CQ"] for g in geos), 128)       # gate width (cols 0..wg)
    maxF = max(g["F"] for g in geos)
    DT_X = fp8 if USE_FP8 else bf16
    DT_SC = fp8 if (USE_FP8 and FP8_SCORES) else bf16
    need_xqb = USE_FP8             # gate chains always run bf16
    CQs = [geos[s // 2]["CQ"] for s in range(BC)]
    NOs = [geos[s // 2]["NO"] for s in range(BC)]
    offq = [sum(CQs[:s]) for s in range(BC)]
    offo = [sum(NOs[:s]) for s in range(BC)]
    SQ, SO = sum(CQs), sum(NOs)

    # per-partition SBUF estimate (bytes) with w1-half0 resident early
    _xb = 1 if USE_FP8 else 2
    _scb = 1 if (USE_FP8 and FP8_SCORES) else 2
    _est = (ND * SQ * _xb + ND * SO * _scb + BC * NL * D * 2     # xq, xo, x
            + (ND * SQ * 2 if USE_FP8 else 0)                    # xqb
            + 3 * ND * D * _xb * (2 if need_mt else 1)           # M (+MT)
            + max(ND * 2 * g["w"] * _scb for g in geos) * 3      # projT
            + max(4 * g["F"] * g["NO"] * 2 for g in geos)        # E (bf16)
            + NC3 * 512 * 2                                      # w1 half 0
            + 60 * 1024)                                         # misc + slack
    early_tail = _est <= 200 * 1024
    _projT_extra = (sum(ND * 2 * g["w"] * _scb for g in geos)
                    - max(ND * 2 * g["w"] * _scb for g in geos)) * 3
    interleave_proj = (_est + _projT_extra) <= 200 * 1024

    xq_d = nc.dram_tensor("xq", [ND, 128, SQ], DT_X, kind="ExternalInput").ap()
    xqb_d = (nc.dram_tensor("xqb", [ND, 128, SQ], bf16,
                            kind="ExternalInput").ap() if need_xqb else None)
    need_xo8 = (DT_SC != DT_X) and any(
        g["have"] and not g["side_q"] for g in geos)
    xo_d = (nc.dram_tensor("xo", [ND, 128, SO], DT_SC, kind="ExternalInput").ap()
            if SO else None)
    xo8_d = (nc.dram_tensor("xo8", [ND, 128, SO], DT_X,
                            kind="ExternalInput").ap()
             if (SO and need_xo8) else None)
    x_d = nc.dram_tensor("x", [BC, L, D], bf16, kind="ExternalInput").ap()
    fb_d = nc.dram_tensor("fbias", [BC, L], bf16, kind="ExternalInput").ap()
    ob_d = nc.dram_tensor("obias", [BC, L], bf16, kind="ExternalInput").ap()
    m_d = [nc.dram_tensor(f"m{p}", [D, D], DT_X, kind="ExternalInput").ap()
           for p in range(3)]
    mt_d = ([nc.dram_tensor(f"mt{p}", [D, D], DT_X, kind="ExternalInput").ap()
             for p in range(3)] if need_mt else None)
    emb_d = nc.dram_tensor("wanom_emb", [128, ND, BC, BC], bf16,
                           kind="ExternalInput").ap()
    w1_d = nc.dram_tensor("w_f1", [NC3, 128, D], bf16, kind="ExternalInput").ap()
    w2_d = nc.dram_tensor("w_f2", [ND, 128, D], bf16, kind="ExternalInput").ap()
    b1_d = nc.dram_tensor("b_f1", [1, D], bf16, kind="ExternalInput").ap()
    b2_d = nc.dram_tensor("b_f2", [1, D], bf16, kind="ExternalInput").ap()
    lng_d = lnb_d = None
    if not ln_trivial:
        lng_d = nc.dram_tensor("ln_g", [BC, D], f32, kind="ExternalInput").ap()
        lnb_d = nc.dram_tensor("ln_b", [BC, D], f32, kind="ExternalInput").ap()
    out_d = nc.dram_tensor("out", [BC, D], f32, kind="ExternalOutput").ap()

    with tile.TileContext(nc) as tc, ExitStack() as ctx:
        const_p = ctx.enter_context(tc.tile_pool(name="const", bufs=1))
        main_p = ctx.enter_context(tc.tile_pool(name="main", bufs=1))
        sm_p = ctx.enter_context(tc.tile_pool(name="small", bufs=2))
        tmp_p = ctx.enter_context(tc.tile_pool(name="tmp", bufs=2))
        ps_big = ctx.enter_context(tc.tile_pool(name="psb", bufs=5, space="PSUM"))
        ps_med = ctx.enter_context(tc.tile_pool(name="psm", bufs=1, space="PSUM"))
        ps_sm = ctx.enter_context(tc.tile_pool(name="pss", bufs=2, space="PSUM"))
        tailA_p = (ctx.enter_context(tc.tile_pool(name="tailA", bufs=1))
                   if early_tail else None)
        es2 = ExitStack()   # E matrices; closed after last r
        e_p = es2.enter_context(tc.tile_pool(name="emat", bufs=1))
        es1 = ExitStack()   # W matrices + projT; closed after last scores
        w_p = es1.enter_context(tc.tile_pool(name="w", bufs=1))
        proj_p = es1.enter_context(tc.tile_pool(name="proj", bufs=1))

        # ---- constants ----
        ones_row = const_p.tile([1, 128], bf16)
        nc.vector.memset(ones_row[:], 1.0)
        ones4 = const_p.tile([1, BC], bf16)
        nc.vector.memset(ones4[:], 1.0)
        eyerows = const_p.tile([1, BC, BC], bf16)
        nc.vector.memset(eyerows[:], 0.0)
        for s in range(BC):
            nc.vector.memset(eyerows[:, s, s : s + 1], 1.0)
        iot_t = const_p.tile([128, 128], mybir.dt.int32)
        nc.gpsimd.iota(iot_t[:], pattern=[[1, 128]], base=0, channel_multiplier=-1)
        ident_f = const_p.tile([128, 128], f32)
        nc.vector.tensor_scalar(ident_f[:], iot_t[:], scalar1=0, scalar2=None,
                                op0=ALU.is_equal)
        ident_b = const_p.tile([128, 128], bf16)
        nc.vector.tensor_copy(ident_b[:], ident_f[:])
        warm_t = const_p.tile([1, 1], f32)
        nc.scalar.sqrt(warm_t[:], ones4[0:1, 0:1])
        # HAM warm-up: dep-free matmuls keep the PE busy from t=0 so the
        # clock gate opens (1.2 -> 2.4 GHz) before the DMA-fed work lands.
        warm_ps = ps_med.tile([128, 64], f32, tag="pm", name="warm_ps")
        for _ in range(90):
            nc.tensor.matmul(warm_ps[:], lhsT=ident_b[:],
                             rhs=ident_b[0:128, 0:64], start=True, stop=True)

        emb_t = const_p.tile([128, ND, BC, BC], bf16)
        nc.scalar.dma_start(emb_t[:], emb_d[:])
        fb_t = const_p.tile([1, BC, L], bf16)
        ob_t = const_p.tile([1, BC, L], bf16)
        for s in range(BC):
            nc.scalar.dma_start(fb_t[:, s, :], fb_d[s : s + 1, :])
            nc.scalar.dma_start(ob_t[:, s, :], ob_d[s : s + 1, :])
        b1_t = const_p.tile([1, D], bf16)
        nc.scalar.dma_start(b1_t[:], b1_d[:])
        b2_t = const_p.tile([1, D], bf16)
        nc.scalar.dma_start(b2_t[:], b2_d[:])
        lng_t = lnb_t = None
        if not ln_trivial:
            lng_t = const_p.tile([BC, D], f32)
            nc.scalar.dma_start(lng_t[:], lng_d[:])
            lnb_t = const_p.tile([BC, D], f32)
            nc.scalar.dma_start(lnb_t[:], lnb_d[:])

        # ---- big inputs ----
        xq_t = main_p.tile([128, ND, SQ], DT_X)
        for k in range(ND):
            nc.sync.dma_start(xq_t[:, k, :], xq_d[k])
        xo_t = None
        if SO:
            xo_t = main_p.tile([128, ND, SO], DT_SC)
            for k in range(ND):
                nc.sync.dma_start(xo_t[:, k, :], xo_d[k])
        xo8_t = xo_t
        if SO and need_xo8:
            xo8_t = main_p.tile([128, ND, SO], DT_X)
            for k in range(ND):
                nc.sync.dma_start(xo8_t[:, k, :], xo8_d[k])
        w_ts = []
        for p in range(3):
            wt = w_p.tile([128, ND, D], DT_X, name=f"w{p}")
            for q in range(4):
                nc.gpsimd.dma_start(
                    wt[:, 2 * q : 2 * q + 2, :],
                    m_d[p][q * 256 : (q + 1) * 256].rearrange(
                        "(k p) c -> p k c", p=128))
            w_ts.append(wt)
        wt_ts = []
        if need_mt:
            for p in range(3):
                wtt = w_p.tile([128, ND, D], DT_X, name=f"wt{p}")
                nc.gpsimd.dma_start(
                    wtt[:], mt_d[p].rearrange("(k p) c -> p k c", p=128)
                )
                wt_ts.append(wtt)
        x_t = main_p.tile([128, BC, NL, D], bf16)
        for s in range(BC):
            nc.sync.dma_start(
                x_t[:, s], x_d[s].rearrange("(t p) d -> p t d", p=128)
            )
        xqb_t = xq_t
        if need_xqb:
            xqb_t = w_p.tile([128, ND, SQ], bf16, name="xqb")
            for k in range(ND):
                nc.sync.dma_start(xqb_t[:, k, :], xqb_d[k])

        def _load_w1_half(pool, half, name):
            w1h = pool.tile([128, NC3, 512], bf16, name=name)
            hs = slice(half * 512, (half + 1) * 512)
            for t in range(3):
                nc.gpsimd.dma_start(
                    w1h[:, t * ND : (t + 1) * ND, :],
                    w1_d[t * ND : (t + 1) * ND, :, hs].rearrange(
                        "k p n -> p k n"),
                )
            return w1h

        w1h_t = [None, None]
        if early_tail:
            w1h_t[0] = _load_w1_half(tailA_p, 0, "w1a")

        G_all = main_p.tile([128, NL, BC, 12], bf16)   # col = t*4 + s
        nc.vector.memset(G_all[:], 0.0)
        gcol = main_p.tile([128, maxF, BC], f32)

        # ---- projections: chains grouped by (p, m) so consecutive matmuls
        # share the stationary weight block (LDWEIGHTS hides under streams);
        # both pairs interleave when SBUF allows.
        DR = mybir.MatmulPerfMode.DoubleRow if USE_FP8 else None
        NK = ND // 2 if USE_FP8 else ND

        def alloc_projT(pr):
            tag_pr = pr if interleave_proj else 0
            return [
                proj_p.tile([128, ND, 2, geos[pr]["w"]], DT_SC,
                            tag=f"pj{p}_{tag_pr}", name=f"pj{p}_{pr}")
                for p in range(3)
            ]

        def emit_proj(prs, projTs):
            units = []
            for pr in prs:
                g = geos[pr]
                if not g["have"]:
                    continue
                poff = offq if g["side_q"] else offo
                assert poff[2 * pr + 1] == poff[2 * pr] + g["w"]
                if 2 * g["w"] <= 512:
                    units.append((pr, None))
                else:
                    units.append((pr, 0))
                    units.append((pr, 1))
            for p in range(3):
                for m in range(ND):
                    msl = slice(m * 128, (m + 1) * 128)
                    chains = []
                    for pr, sp in units:
                        w = geos[pr]["w"]
                        shape = [128, 2, w] if sp is None else [128, w]
                        ps = ps_big.tile(shape, f32, tag="ps",
                                         name=f"pjps{pr}")
                        chains.append((pr, sp, ps))
                    for k in range(NK):
                        for pr, sp, ps in chains:
                            g = geos[pr]
                            xsrc = xq_t if g["side_q"] else xo8_t
                            poff = offq if g["side_q"] else offo
                            wm = (w_ts if g["side_q"] else wt_ts)[p]
                            off = poff[2 * pr + (sp or 0)]
                            wid = 2 * g["w"] if sp is None else g["w"]
                            if USE_FP8:
                                nc.tensor.matmul(
                                    ps[:],
                                    lhsT=wm[:, 2 * k : 2 * k + 2, msl],
                                    rhs=xsrc[:, 2 * k : 2 * k + 2,
                                             off : off + wid],
                                    start=(k == 0), stop=(k == NK - 1),
                                    perf_mode=DR,
                                )
                            else:
                                nc.tensor.matmul(
                                    ps[:], lhsT=wm[:, k, msl],
                                    rhs=xsrc[:, k, off : off + wid],
                                    start=(k == 0), stop=(k == NK - 1),
                                )
                    for i, (pr, sp, ps) in enumerate(chains):
                        dst = (projTs[pr][p][:, m] if sp is None
                               else projTs[pr][p][:, m, sp])
                        if (m + i) % 2 == 0:
                            nc.vector.tensor_copy(dst, ps[:])
                        else:
                            nc.scalar.activation(dst, ps[:], AF.Copy)

        projTs = {}
        if interleave_proj:
            for pr in range(2):
                if geos[pr]["have"]:
                    projTs[pr] = alloc_projT(pr)
            emit_proj([0, 1], projTs)

        # ---- gates (all samples): logits in ONE [4, wg] PSUM ----
        ga_ps = ps_med.tile([BC, wg], f32, tag="pm")
        for k in range(ND):
            for s in range(BC):
                nc.tensor.matmul(
                    ga_ps[:, 0 : CQs[s]], lhsT=emb_t[:, k, s, :],
                    rhs=xqb_t[:, k, offq[s] : offq[s] + CQs[s]],
                    start=(k == 0 and s == 0), stop=False,
                )
        for s in range(BC):
            nc.tensor.matmul(
                ga_ps[:], lhsT=eyerows[:, s, :], rhs=fb_t[:, s, 0:wg],
                start=False, stop=(s == BC - 1),
            )
        grow = sm_p.tile([BC, wg], f32, tag="grow", bufs=1)
        gs_t = sm_p.tile([BC, 1], f32, tag="gs", bufs=1)
        nc.scalar.activation(grow[:], ga_ps[:], AF.Exp, scale=1.0 / PRE,
                             accum_out=gs_t[:])
        nc.vector.tensor_scalar_max(gs_t[:], gs_t[:], 1e-8)
        rg_t = sm_p.tile([BC, 1], f32, tag="rg", bufs=1)
        nc.vector.reciprocal(rg_t[:], gs_t[:])
        nc.vector.tensor_scalar_mul(grow[:], grow[:], rg_t[:])
        for it in range(maxF):
            gt_ps = ps_med.tile([128, BC], f32, tag="pm")
            nc.tensor.transpose(gt_ps[:], grow[:, it * 128 : (it + 1) * 128],
                                ident_f[0:BC, 0:BC])
            nc.vector.tensor_copy(gcol[:, it, :], gt_ps[:])
            for s in range(BC):
                if it < geos[s // 2]["F"]:
                    nc.vector.tensor_copy(G_all[:, it, s, s : s + 1],
                                          gt_ps[:, s : s + 1])

        # ---- per pair: (projections) -> scores -> E -> coeffs -> r ----
        for pr in range(2):
            g = geos[pr]
            F, OJ, NO, w, lo = g["F"], g["OJ"], g["NO"], g["w"], g["lo"]
            if not g["have"]:
                continue
            s0, s1 = 2 * pr, 2 * pr + 1
            if not interleave_proj:
                projTs[pr] = alloc_projT(pr)
                emit_proj([pr], projTs)
            projT = projTs[pr]

            for sp in range(2):
                s4 = s0 + sp
                E_sup = e_p.tile([128, F, NO], bf16, tag=f"Es{sp}",
                                 name=f"Es{sp}_{pr}")
                E_rep = e_p.tile([128, F, NO], bf16, tag=f"Er{sp}",
                                 name=f"Er{sp}_{pr}")
                co_sup = sm_p.tile([128, F], bf16, tag=f"cos{sp}", bufs=1,
                                   name=f"cos{sp}_{pr}")
                co_rep = sm_p.tile([128, F], bf16, tag=f"cor{sp}", bufs=1,
                                   name=f"cor{sp}_{pr}")
                sc_dr = USE_FP8 and FP8_SCORES
                NKS = ND // 2 if sc_dr else ND
                for it in range(F):
                    isl = slice(it * 128, (it + 1) * 128)
                    ps3 = {}
                    for p in (0, 2, 1):   # sup, rep, con
                        ps = ps_big.tile([128, NO], f32, tag="ps",
                                         name=f"sc{p}")
                        ps3[p] = ps
                        for k in range(NKS):
                            if sc_dr:
                                ksl = slice(2 * k, 2 * k + 2)
                                if g["side_q"]:
                                    lhsT = projT[p][:, ksl, sp, isl]
                                    rhs = xo_t[:, ksl,
                                               offo[s4] : offo[s4] + NO]
                                else:
                                    lhsT = xq_t[:, ksl,
                                                offq[s4] + it * 128 :
                                                offq[s4] + (it + 1) * 128]
                                    rhs = projT[p][:, ksl, sp, 0:NO]
                                nc.tensor.matmul(
                                    ps[:], lhsT=lhsT, rhs=rhs,
                                    start=(k == 0),
                                    stop=(k == NKS - 1 and p == 1),
                                    perf_mode=mybir.MatmulPerfMode.DoubleRow,
                                )
                            else:
                                if g["side_q"]:
                                    lhsT = projT[p][:, k, sp, isl]
                                    rhs = xo_t[:, k, offo[s4] : offo[s4] + NO]
                                else:
                                    lhsT = xqb_t[:, k,
                                                 offq[s4] + it * 128 :
                                                 offq[s4] + (it + 1) * 128]
                                    rhs = projT[p][:, k, sp, 0:NO]
                                nc.tensor.matmul(ps[:], lhsT=lhsT, rhs=rhs,
                                                 start=(k == 0),
                                                 stop=(k == NKS - 1 and p == 1))
                        if p != 1:
                            # option-mask bias row closes the sup/rep groups
                            nc.tensor.matmul(ps[:], lhsT=ones_row[:],
                                             rhs=ob_t[:, s4, OJ:L],
                                             start=False, stop=True)
                    ps_sup, ps_rep, ps_con = ps3[0], ps3[2], ps3[1]

                    T_t = tmp_p.tile([128, NO], f32, tag="T")
                    nc.scalar.activation(T_t[:], ps_con[:], AF.Tanh,
                                         scale=SCALE / PRE)
                    A_t = tmp_p.tile([128, NO], f32, tag="A")
                    nc.vector.scalar_tensor_tensor(
                        A_t[:], in0=ps_rep[:], scalar=SCALE / PRE, in1=T_t[:],
                        op0=ALU.mult, op1=ALU.add,
                    )
                    rs_sup = sm_p.tile([128, 1], f32, tag="rss")
                    nc.scalar.activation(E_sup[:, it], ps_sup[:], AF.Exp,
                                         scale=SCALE / PRE, accum_out=rs_sup[:])
                    rs_rep = sm_p.tile([128, 1], f32, tag="rsr")
                    nc.scalar.activation(E_rep[:, it], A_t[:], AF.Exp,
                                         accum_out=rs_rep[:])
                    rc_sup = sm_p.tile([128, 1], f32, tag="rcs")
                    nc.vector.reciprocal(rc_sup[:], rs_sup[:])
                    nc.vector.tensor_mul(co_sup[:, it : it + 1],
                                         gcol[:, it, s4 : s4 + 1], rc_sup[:])
                    rc_rep = sm_p.tile([128, 1], f32, tag="rcr")
                    nc.vector.reciprocal(rc_rep[:], rs_rep[:])
                    nc.vector.tensor_mul(co_rep[:, it : it + 1],
                                         gcol[:, it, s4 : s4 + 1], rc_rep[:])

                # r vectors: G col 4+s (rep), 8+s (sup)
                for t, (E_t, co_t) in enumerate(((E_rep, co_rep),
                                                 (E_sup, co_sup))):
                    for jt in range(NO // 128):
                        jsl = slice(jt * 128, (jt + 1) * 128)
                        r_ps = ps_sm.tile([128, 1], f32, tag="r")
                        for it in range(F):
                            nc.tensor.matmul(
                                r_ps[:], lhsT=E_t[:, it, jsl],
                                rhs=co_t[:, it : it + 1],
                                start=(it == 0), stop=(it == F - 1),
                            )
                        nc.vector.tensor_copy(
                            G_all[:, g["J0"] + jt, s4,
                                  4 * (t + 1) + s4 : 4 * (t + 1) + s4 + 1],
                            r_ps[:],
                        )

        es1.close()
        es2.close()

        # ---- remaining tail weights (after proj/W pools freed) ----
        tail_p = ctx.enter_context(tc.tile_pool(name="tail", bufs=1))
        if not early_tail:
            w1h_t[0] = _load_w1_half(tail_p, 0, "w1a2")
        w1h_t[1] = _load_w1_half(tail_p, 1, "w1b")
        w2_t = tail_p.tile([128, ND, D], bf16, name="w2sb")
        nc.gpsimd.dma_start(w2_t[:], w2_d.rearrange("k p n -> p k n"))

        # ---- pool: pooled[t*4+s, :] = sum_l G[l, t*4+s] * x_s[l, :] ----
        seq = []
        for s4 in range(BC):
            g = geos[s4 // 2]
            rts = sorted(set(range(g["F"]))
                         | (set(range(g["J0"], NL)) if g["have"] else set()))
            seq.extend((s4, rt) for rt in rts)
        pooled_sb = main_p.tile([12, D], bf16)
        fused_sb = main_p.tile([128, ND, 3, BC], bf16)
        for half in range(2):
            hs = slice(half * 512, (half + 1) * 512)
            po_ps = ps_med.tile([12, 512], f32, tag="pm")
            for i, (s4, rt) in enumerate(seq):
                nc.tensor.matmul(
                    po_ps[:], lhsT=G_all[:, rt, s4, :], rhs=x_t[:, s4, rt, hs],
                    start=(i == 0), stop=(i == len(seq) - 1),
                )
            nc.vector.tensor_copy(pooled_sb[:, hs], po_ps[:])
            for m in range(half * 4, half * 4 + 4):
                tr_ps = ps_sm.tile([128, 12], bf16, tag="r")
                nc.tensor.transpose(tr_ps[:],
                                    pooled_sb[:, m * 128 : (m + 1) * 128],
                                    ident_b[0:12, 0:12])
                nc.vector.tensor_copy(fused_sb[:, m], tr_ps[:])

        # ---- MLP tail + LayerNorm, all in [4, 1024] row form ----
        h_sb = main_p.tile([BC, D], bf16)
        hT_sb = main_p.tile([128, ND, BC], bf16)
        for half in range(2):
            hs = slice(half * 512, (half + 1) * 512)
            h_ps = ps_big.tile([BC, 512], f32, tag="ps")
            for t in range(3):
                for m in range(ND):
                    nc.tensor.matmul(
                        h_ps[:], lhsT=fused_sb[:, m, t, :],
                        rhs=w1h_t[half][:, t * ND + m, :],
                        start=(t == 0 and m == 0), stop=False,
                    )
            nc.tensor.matmul(h_ps[:], lhsT=ones4[:], rhs=b1_t[:, hs],
                             start=False, stop=True)
            nc.scalar.activation(h_sb[:, hs], h_ps[:], AF.Relu)
        for m in range(ND):
            ht_ps = ps_sm.tile([128, BC], bf16, tag="r")
            nc.tensor.transpose(ht_ps[:], h_sb[:, m * 128 : (m + 1) * 128],
                                ident_b[0:BC, 0:BC])
            nc.vector.tensor_copy(hT_sb[:, m], ht_ps[:])

        y_sb = main_p.tile([BC, D], f32)
        s1_t = sm_p.tile([BC, 2], f32, tag="s1", bufs=1)
        s2_t = sm_p.tile([BC, 2], f32, tag="s2", bufs=1)
        for half in range(2):
            hs = slice(half * 512, (half + 1) * 512)
            y_ps = ps_big.tile([BC, 512], f32, tag="ps")
            for k in range(ND):
                nc.tensor.matmul(y_ps[:], lhsT=hT_sb[:, k, :],
                                 rhs=w2_t[:, k, hs],
                                 start=(k == 0), stop=False)
            nc.tensor.matmul(y_ps[:], lhsT=ones4[:], rhs=b2_t[:, hs],
                             start=False, stop=True)
            if half == 0:
                nc.scalar.activation(y_sb[:, hs], y_ps[:], AF.Copy,
                                     accum_out=s1_t[:, half : half + 1])
            else:
                # split across engines: DVE copies+sums while ACT squares
                nc.vector.tensor_copy(y_sb[:, hs], y_ps[:])
                nc.vector.tensor_reduce(s1_t[:, half : half + 1],
                                        y_ps[:], axis=mybir.AxisListType.X,
                                        op=ALU.add)
            sq = tmp_p.tile([BC, 512], f32, tag="sq", bufs=2)
            nc.scalar.activation(sq[:], y_ps[:], AF.Square,
                                 accum_out=s2_t[:, half : half + 1])

        mu_t = sm_p.tile([BC, 1], f32, tag="mu", bufs=1)
        nc.vector.tensor_reduce(mu_t[:], s1_t[:], axis=mybir.AxisListType.X,
                                op=ALU.add)
        nc.scalar.mul(mu_t[:], mu_t[:], 1.0 / D)
        msq_t = sm_p.tile([BC, 1], f32, tag="msq", bufs=1)
        nc.vector.tensor_reduce(msq_t[:], s2_t[:], axis=mybir.AxisListType.X,
                                op=ALU.add)
        nc.scalar.mul(msq_t[:], msq_t[:], 1.0 / D)
        m2_t = sm_p.tile([BC, 1], f32, tag="m2", bufs=1)
        nc.vector.tensor_mul(m2_t[:], mu_t[:], mu_t[:])
        var_t = sm_p.tile([BC, 1], f32, tag="var", bufs=1)
        nc.vector.tensor_scalar(var_t[:], msq_t[:], scalar1=m2_t[:],
                                scalar2=LN_EPS, op0=ALU.subtract, op1=ALU.add)
        sd_t = sm_p.tile([BC, 1], f32, tag="sd", bufs=1)
        nc.scalar.sqrt(sd_t[:], var_t[:])
        rstd_t = sm_p.tile([BC, 1], f32, tag="rstd", bufs=1)
        nc.vector.reciprocal(rstd_t[:], sd_t[:])
        nmr_t = sm_p.tile([BC, 1], f32, tag="nmr", bufs=1)
        nc.vector.tensor_scalar(nmr_t[:], mu_t[:], scalar1=rstd_t[:],
                                scalar2=-1.0, op0=ALU.mult, op1=ALU.mult)

        z_sb = main_p.tile([BC, D], f32)
        for half in range(2):
            hs = slice(half * 512, (half + 1) * 512)
            if half == 0:
                nc.scalar.activation(z_sb[:, hs], y_sb[:, hs], AF.Identity,
                                     scale=rstd_t[:], bias=nmr_t[:])
            else:
                # other engine: runs concurrently with half 0 + its DMA
                nc.vector.tensor_scalar(z_sb[:, hs], y_sb[:, hs],
                                        scalar1=mu_t[:], scalar2=rstd_t[:],
                                        op0=ALU.subtract, op1=ALU.mult)
            if not ln_trivial:
                nc.vector.tensor_mul(z_sb[:, hs], z_sb[:, hs], lng_t[:, hs])
                nc.vector.tensor_add(z_sb[:, hs], z_sb[:, hs], lnb_t[:, hs])
            nc.sync.dma_start(out_d[:, hs], z_sb[:, hs])

    nc.compile()
    return nc


def _masks(x_ids, pad_idx, sep_idx):
    valid = x_ids != pad_idx
    sepm = x_ids == sep_idx
    has = sepm.any(axis=1)
    first = sepm.argmax(axis=1)
    vlen = valid.sum(axis=1)
    fb = np.clip(vlen // 2, 1, max(1, L - 2))
    sp = np.where(has, first, fb)
    pos = np.arange(L)
    fmask = (pos[None, :] < sp[:, None]) & valid
    omask = (pos[None, :] > sp[:, None]) & valid
    return sp, fmask, omask


def _host_prep_fast(inputs):
    import os

    x = np.asarray(inputs["x"], dtype=np.float32)
    x_ids = np.asarray(inputs["x_ids"])
    pad_idx = int(np.asarray(inputs["pad_idx"]))
    sep_idx = int(np.asarray(inputs["sep_idx"]))
    assert x.shape == (B, L, D), x.shape
    np_x = np_fp8 if USE_FP8 else np_bf16

    sp, fmask, omask = _masks(x_ids, pad_idx, sep_idx)
    fb = np.where(fmask, 0.0, FBIAS * PRE).astype(np_bf16)
    ob = np.where(omask, 0.0, OBIAS_RAW * PRE).astype(np_bf16)

    order = np.argsort(-sp, kind="stable")
    F_all = np.maximum(np.ceil(sp / 128).astype(int), 1)
    J0_all = np.minimum((sp + 1) // 128, NL)
    pair_geo = tuple(
        (int(F_all[order[pr * 16 : (pr + 1) * 16]].max()),
         int(J0_all[order[pr * 16 : (pr + 1) * 16]].min()))
        for pr in range(2)
    )
    fbnd = os.environ.get("FORCE_BOUNDS")
    if fbnd:
        f0, j0, f1, j1 = (int(v) for v in fbnd.split(","))
        pair_geo = ((f0, j0), (f1, j1))
    geos = [_geo(F, J0) for (F, J0) in pair_geo]
    need_mt = any(g["have"] and not g["side_q"] for g in geos)
    np_sc = np_fp8 if (USE_FP8 and FP8_SCORES) else np_bf16
    has_kside = any(g["have"] and not g["side_q"] for g in geos)
    need_xqb = USE_FP8
    need_xo8 = (np_sc != np_x) and has_kside
    CQs = [geos[s // 2]["CQ"] for s in range(BC)]
    NOs = [geos[s // 2]["NO"] for s in range(BC)]
    OJs = [geos[s // 2]["OJ"] for s in range(BC)]
    SQ, SO = sum(CQs), sum(NOs)

    def w(name):
        return np.ascontiguousarray(np.asarray(inputs[name], dtype=np.float32))

    shared = {}
    for p, (qn, kn) in enumerate((("w_sq", "w_sk"), ("w_cq", "w_ck"),
                                  ("w_rq", "w_rk"))):
        shared[f"m{p}"] = _m_matrix(inputs[qn], inputs[kn])
        if need_mt:
            shared[f"mt{p}"] = _m_matrix(inputs[qn], inputs[kn], transposed=True)

    wanom_pm = w("w_anom").reshape(ND, 128).T            # [128, ND]
    emb = np.zeros((128, ND, BC, BC), np.float32)
    for s in range(BC):
        emb[:, :, s, s] = wanom_pm * PRE
    shared["wanom_emb"] = emb.astype(np_bf16)

    shared["w_f1"] = np.ascontiguousarray(
        w("w_f1").reshape(NC3, 128, D)).astype(np_bf16)
    shared["w_f2"] = np.ascontiguousarray(
        w("w_f2").reshape(ND, 128, D)).astype(np_bf16)
    shared["b_f1"] = w("b_f1").reshape(1, D).astype(np_bf16)
    shared["b_f2"] = w("b_f2").reshape(1, D).astype(np_bf16)
    ln_g, ln_b = w("ln_g"), w("ln_b")
    ln_trivial = bool(np.all(ln_g == 1.0) and np.all(ln_b == 0.0))
    if not ln_trivial:
        shared["ln_g"] = np.broadcast_to(ln_g.reshape(1, D), (BC, D)).copy()
        shared["ln_b"] = np.broadcast_to(ln_b.reshape(1, D), (BC, D)).copy()

    in_maps = []
    core_idx = []
    for c in range(NCORES):
        idx = order[np.arange(BC) * NCORES + c]
        core_idx.append(idx)
        xs = x[idx]                                      # [BC, L, D] f32
        m = dict(shared)
        m["x"] = xs.astype(np_bf16)
        xsT = np.ascontiguousarray(xs.transpose(2, 0, 1))   # [D, BC, L] f32
        xq_f = np.ascontiguousarray(np.concatenate(
            [xsT[:, s, 0 : CQs[s]] for s in range(BC)], axis=1,
        ))
        m["xq"] = xq_f.reshape(ND, 128, SQ).astype(np_x)
        if need_xqb:
            m["xqb"] = xq_f.reshape(ND, 128, SQ).astype(np_bf16)
        if SO:
            xo_f = np.ascontiguousarray(np.concatenate(
                [xsT[:, s, OJs[s] : L] for s in range(BC)], axis=1,
            ))
            m["xo"] = xo_f.reshape(ND, 128, SO).astype(np_sc)
            if need_xo8:
                m["xo8"] = xo_f.reshape(ND, 128, SO).astype(np_x)
        m["fbias"] = np.ascontiguousarray(fb[idx])
        m["obias"] = np.ascontiguousarray(ob[idx])
        in_maps.append(m)
    return in_maps, (pair_geo, ln_trivial), core_idx


def get_program_fast(key):
    if key not in _PROGRAM_CACHE:
        pair_geo, ln_trivial = key
        _PROGRAM_CACHE[key] = build_program_fast(pair_geo, ln_trivial)
    return _PROGRAM_CACHE[key]


def run(trace=False, **inputs):
    use_m = all(
        not np.any(np.asarray(inputs[n]))
        for n in ("b_sq", "b_sk", "b_cq", "b_ck", "b_rq", "b_rk")
    )
    if not use_m:
        return _run_legacy(trace=trace, **inputs)
    in_maps, key, core_idx = _host_prep_fast(inputs)
    nc = get_program_fast(key)
    res = bass_utils.run_bass_kernel_spmd(
        nc, in_maps, core_ids=list(range(NCORES)), trace=trace
    )
    out = np.empty((B, D), np.float32)
    for c in range(NCORES):
        out[core_idx[c]] = res.results[c]["out"]
    return out, res


def kernel(**inputs):
    out, _ = run(trace=False, **inputs)
    return out


# ---------------------------------------------------------------------------
# Fallback (nonzero projection biases): exact numpy reference. This path is
# not expected in practice (setup_inputs uses zero biases); correctness over
# speed.
# ---------------------------------------------------------------------------

NEG = -9.0e15


def _run_legacy(trace=False, **inputs):
    x = np.asarray(inputs["x"], dtype=np.float32)
    x_ids = np.asarray(inputs["x_ids"])
    pad_idx = int(np.asarray(inputs["pad_idx"]))
    sep_idx = int(np.asarray(inputs["sep_idx"]))

    def w(name):
        return np.asarray(inputs[name], dtype=np.float32)

    _, fmask, omask = _masks(x_ids, pad_idx, sep_idx)

    al = (x @ w("w_anom") + w("b_anom"))[..., 0]
    al = np.where(fmask, al, NEG)
    al -= al.max(axis=1, keepdims=True)
    gate = np.exp(al)
    gate /= gate.sum(axis=1, keepdims=True)
    gate = gate * fmask
    gate = gate / np.clip(gate.sum(axis=1, keepdims=True), 1e-8, None)

    scale = 1.0 / np.sqrt(D)
    pair = fmask[:, :, None] & omask[:, None, :]

    def attn(sq, bq, sk, bk, extra=None):
        q = x @ w(sq) + w(bq)
        k = x @ w(sk) + w(bk)
        s = np.einsum("bid,bjd->bij", q, k) * scale
        if extra is not None:
            s = s + extra
        s = np.where(pair, s, NEG)
        s -= s.max(axis=2, keepdims=True)
        e = np.exp(s)
        return e / e.sum(axis=2, keepdims=True)

    qc = x @ w("w_cq") + w("b_cq")
    kc = x @ w("w_ck") + w("b_ck")
    conf = np.tanh(np.einsum("bid,bjd->bij", qc, kc) * scale)
    sup_a = attn("w_sq", "b_sq", "w_sk", "b_sk")
    rep_a = attn("w_rq", "b_rq", "w_rk", "b_rk", extra=conf)

    rep_vec = np.einsum("bij,bjd->bid", rep_a, x)
    sup_vec = np.einsum("bij,bjd->bid", sup_a, x)
    fused = np.concatenate([
        np.einsum("bl,bld->bd", gate, x),
        np.einsum("bl,bld->bd", gate, rep_vec),
        np.einsum("bl,bld->bd", gate, sup_vec),
    ], axis=-1)
    fused = np.maximum(fused @ w("w_f1") + w("b_f1"), 0.0) @ w("w_f2") + w("b_f2")
    mu = fused.mean(axis=-1, keepdims=True)
    var = fused.var(axis=-1, keepdims=True)
    fused = (fused - mu) / np.sqrt(var + LN_EPS) * w("ln_g") + w("ln_b")

    class _Res:
        results = None
        exec_time_ns = None

    return fused.astype(np.float32), _Res()
